# revision 10
# baseline (speedup 1.0000x reference)
"""Trainium2 Bass kernel for ConfidenceMaskedDecoder.

Strategy (8 NeuronCores, data-parallel over the B*S=8192 rows, 1024 rows/core):
  Device, per core (rows r = token positions, V=32000 vocab, E=2048 hidden):
    * Stream logits [1024, 32000] f32 through SBUF in [128, 2000] chunks:
        - ACT: exp(chunk) -> bf16 tile, fused accumulate-sum -> per-row sumexp
        - DVE: per-chunk max of the bf16 exps (2x mode), then find-index of
          that max within the chunk (InstMaxIndex)
        - tiny per-group combines give per-row max(exp) and global argmax
      max softmax prob = max(exp(l)) / sum(exp(l)); no max-subtraction is
      needed (|logits| <= ~6 here so exp cannot overflow in fp32).
    * Confidence head on PE: out1^T[f, r] = W1^T.T @ hidden^T (accumulate over
      E in 16 K-chunks of 128), ACT Gelu(+b1) -> h^T, then
      x2[1, r] = W2^T.T @ h^T accumulated over the 8 f-chunks.
  Host: only O(B*S) epilogue — sigmoid, confidence mix, threshold/fallback
  mask update, token scatter — mirroring the reference in float32 numpy.
"""

import os
import time

import numpy as np

_P = 128
_B, _S, _V, _E = 4, 2048, 32000, 2048
_F = _E // 2  # 1024
_NC = 8  # cores
_RT = _B * _S  # 8192 rows total
_R = _RT // _NC  # 1024 rows per core
_G = _R // _P  # 8 row groups per core
_CV = 2000  # vocab chunk
_NCH = _V // _CV  # 16 chunks
_NR = 512  # rows per matmul tile (PSUM free dim)
_NN = _R // _NR  # 2
_KE = _E // _P  # 16 contraction chunks
_FC = _F // _P  # 8 feature chunks

_THRESHOLD = np.float32(0.8)

# fp32 matmuls: 4 cycles/row on PE but bit-accurate enough that the
# argmax-over-S of conf matches the fp32 reference.  "float32r" runs 4x
# faster; flipped on after HW validation (see MM_DTYPE_ENV).
_MM_DTYPE = os.environ.get("KERNEL_MM_DTYPE", "float32r")

_nc_cache = {}
last_exec_times = None  # list of per-rep seconds for the last device run


def _build_nc():
    import concourse.bacc as bacc
    import concourse.mybir as mybir
    import concourse.tile as tile

    f32 = mybir.dt.float32
    bf16 = mybir.dt.bfloat16
    u32 = mybir.dt.uint32
    mmdt = getattr(mybir.dt, _MM_DTYPE)
    AF = mybir.ActivationFunctionType
    ALU = mybir.AluOpType
    AX = mybir.AxisListType

    nc = bacc.Bacc("TRN2", target_bir_lowering=False, debug=False, num_devices=_NC)
    lg = nc.dram_tensor("lg", [_R, _V], f32, kind="ExternalInput").ap()
    ht = nc.dram_tensor("ht", [_E, _R], mmdt, kind="ExternalInput").ap()
    w1t = nc.dram_tensor("w1t", [_E, _F], mmdt, kind="ExternalInput").ap()
    b1v = nc.dram_tensor("b1v", [_F], f32, kind="ExternalInput").ap()
    w2t = nc.dram_tensor("w2t", [_F], mmdt, kind="ExternalInput").ap()
    o_sum = nc.dram_tensor("o_sum", [_G, _P], f32, kind="ExternalOutput").ap()
    o_max = nc.dram_tensor("o_max", [_G, _P], f32, kind="ExternalOutput").ap()
    o_arg = nc.dram_tensor("o_arg", [_G, _P], f32, kind="ExternalOutput").ap()
    o_x2 = nc.dram_tensor("o_x2", [1, _R], f32, kind="ExternalOutput").ap()

    with tile.TileContext(nc) as tc:
        with (
            tc.tile_pool(name="consts", bufs=1) as consts,
            tc.tile_pool(name="outacc", bufs=1) as outacc,
            tc.tile_pool(name="htp", bufs=2) as htp,
            tc.tile_pool(name="hgp", bufs=1) as hgp,
            tc.tile_pool(name="lgp", bufs=4) as lgp,
            tc.tile_pool(name="exq", bufs=4) as exq,
            tc.tile_pool(name="stats", bufs=2) as stats,
            tc.tile_pool(name="small", bufs=4) as small,
            tc.tile_pool(name="ps1", bufs=6, space="PSUM") as ps1p,
            tc.tile_pool(name="ps2", bufs=2, space="PSUM") as ps2p,
        ):
            # ---- replicated constants ----
            w1t_sb = consts.tile([_P, _KE, _F], mmdt)
            nc.sync.dma_start(out=w1t_sb[:], in_=w1t.rearrange("(k p) f -> p k f", p=_P))
            b1_sb = consts.tile([_P, _FC], f32)
            nc.sync.dma_start(out=b1_sb[:], in_=b1v.rearrange("(c p) -> p c", p=_P))
            w2t_sb = consts.tile([_P, _FC], mmdt)
            nc.sync.dma_start(out=w2t_sb[:], in_=w2t.rearrange("(c p) -> p c", p=_P))
            offs = consts.tile([_P, _NCH], f32)
            for c in range(_NCH):
                nc.vector.memset(offs[:, c : c + 1], float(c * _CV))
            bigc = consts.tile([_P, _NCH], f32)
            nc.vector.memset(bigc[:], 1.0e9)

            osum_sb = outacc.tile([_P, _G], f32)
            omax_sb = outacc.tile([_P, _G], f32)
            oarg_sb = outacc.tile([_P, _G], f32)
            x2_sb = outacc.tile([1, _R], f32)

            # ---- logits streaming: sumexp, max(exp), argmax ----
            for g in range(_G):
                sech = stats.tile([_P, _NCH], f32, tag="sech")
                cmax = stats.tile([_P, _NCH], bf16, tag="cmax")
                fsc = stats.tile([_P, _NCH, 8], u32, tag="fsc")
                for c in range(_NCH):
                    lt = lgp.tile([_P, _CV], f32, tag="lt")
                    nc.sync.dma_start(
                        out=lt[:], in_=lg[g * _P : (g + 1) * _P, c * _CV : (c + 1) * _CV]
                    )
                    et = exq.tile([_P, _CV], bf16, tag="et")
                    nc.scalar.activation(
                        out=et[:], in_=lt[:], func=AF.Exp, accum_out=sech[:, c : c + 1]
                    )
                    nc.vector.tensor_reduce(
                        out=cmax[:, c : c + 1], in_=et[:], axis=AX.X, op=ALU.max
                    )
                    nc.vector.max_index(
                        out=fsc[:, c, :],
                        in_max=cmax[:, c : c + 1].to_broadcast([_P, 8]),
                        in_values=et[:],
                    )
                gmaxb = small.tile([_P, 1], bf16, tag="gmaxb")
                nc.vector.tensor_reduce(out=gmaxb[:], in_=cmax[:], axis=AX.X, op=ALU.max)
                nc.vector.tensor_copy(out=omax_sb[:, g : g + 1], in_=gmaxb[:])
                nc.vector.tensor_reduce(
                    out=osum_sb[:, g : g + 1], in_=sech[:], axis=AX.X, op=ALU.add
                )
                cidx = small.tile([_P, _NCH], f32, tag="cidx")
                nc.vector.tensor_copy(out=cidx[:], in_=fsc[:, :, 0])
                nc.vector.tensor_tensor(out=cidx[:], in0=cidx[:], in1=offs[:], op=ALU.add)
                eq = small.tile([_P, _NCH], u32, tag="eq")
                nc.vector.tensor_scalar(
                    out=eq[:],
                    in0=cmax[:],
                    scalar1=omax_sb[:, g : g + 1],
                    scalar2=None,
                    op0=ALU.is_equal,
                )
                cand = small.tile([_P, _NCH], f32, tag="cand")
                nc.vector.select(out=cand[:], mask=eq[:], on_true=cidx[:], on_false=bigc[:])
                nc.vector.tensor_reduce(
                    out=oarg_sb[:, g : g + 1], in_=cand[:], axis=AX.X, op=ALU.min
                )

            # ---- confidence-head MLP ----
            ht_r = ht.rearrange("(k p) r -> p k r", p=_P)
            for n in range(_NN):
                ht_t = htp.tile([_P, _KE, _NR], mmdt, tag="ht")
                nc.sync.dma_start(out=ht_t[:], in_=ht_r[:, :, n * _NR : (n + 1) * _NR])
                hg = hgp.tile([_P, _FC, _NR], mmdt, tag="hg")
                for fb in range(2):
                    pstiles = [
                        ps1p.tile([_P, _NR], f32, tag="ps1", name=f"ps1_{n}_{fb}_{i}")
                        for i in range(4)
                    ]
                    for ff in range(4):
                        fc = fb * 4 + ff
                        for k in range(_KE):
                            nc.tensor.matmul(
                                pstiles[ff][:],
                                lhsT=w1t_sb[:, k, fc * _P : (fc + 1) * _P],
                                rhs=ht_t[:, k, :],
                                start=(k == 0),
                                stop=(k == _KE - 1),
                            )
                    for ff in range(4):
                        fc = fb * 4 + ff
                        nc.scalar.activation(
                            out=hg[:, fc, :],
                            in_=pstiles[ff][:],
                            func=AF.Gelu,
                            bias=b1_sb[:, fc : fc + 1],
                            scale=1.0,
                        )
                ps2 = ps2p.tile([1, _NR], f32, tag="ps2")
                for fc in range(_FC):
                    nc.tensor.matmul(
                        ps2[:],
                        lhsT=w2t_sb[:, fc : fc + 1],
                        rhs=hg[:, fc, :],
                        start=(fc == 0),
                        stop=(fc == _FC - 1),
                    )
                nc.scalar.copy(out=x2_sb[0:1, n * _NR : (n + 1) * _NR], in_=ps2[:])

            nc.sync.dma_start(out=o_sum.rearrange("g p -> p g"), in_=osum_sb[:])
            nc.sync.dma_start(out=o_max.rearrange("g p -> p g"), in_=omax_sb[:])
            nc.sync.dma_start(out=o_arg.rearrange("g p -> p g"), in_=oarg_sb[:])
            nc.sync.dma_start(out=o_x2[:], in_=x2_sb[:])

    nc.compile()
    return nc


def _get_nc():
    if "nc" not in _nc_cache:
        _nc_cache["nc"] = _build_nc()
    return _nc_cache["nc"]


def _run_device(in_maps, reps=1):
    """Run the per-core kernel on the 8 NeuronCores.  Modeled on
    concourse.bass2jax.run_bass_via_pjrt, with input pre-staging so repeated
    executions time the NEFF itself rather than host->device transfer."""
    global last_exec_times
    import jax
    import concourse.mybir as mybir
    from jax.experimental.shard_map import shard_map
    from jax.sharding import Mesh, NamedSharding, PartitionSpec
    from concourse import bass2jax

    nc = _get_nc()
    bass2jax.install_neuronx_cc_hook()

    partition_name = nc.partition_id_tensor.name if nc.partition_id_tensor else None
    in_names, out_names, out_avals = [], [], []
    for alloc in nc.m.functions[0].allocations:
        if not isinstance(alloc, mybir.MemoryLocationSet):
            continue
        name = alloc.memorylocations[0].name
        if alloc.kind == "ExternalInput":
            if name != partition_name:
                in_names.append(name)
        elif alloc.kind == "ExternalOutput":
            out_names.append(name)
            out_avals.append(
                jax.core.ShapedArray(tuple(alloc.tensor_shape), mybir.dt.np(alloc.dtype))
            )
    n_params = len(in_names)
    n_outs = len(out_names)
    all_names = in_names + out_names
    if partition_name is not None:
        all_names = all_names + [partition_name]

    def _body(*args):
        operands = list(args)
        if partition_name is not None:
            operands.append(bass2jax.partition_id_tensor())
        outs = bass2jax._bass_exec_p.bind(
            *operands,
            out_avals=tuple(out_avals),
            in_names=tuple(all_names),
            out_names=tuple(out_names),
            lowering_input_output_aliases=(),
            sim_require_finite=True,
            sim_require_nnan=True,
            nc=nc,
        )
        return tuple(outs)

    devices = jax.devices()[:_NC]
    mesh = Mesh(np.asarray(devices), ("core",))
    sharding = NamedSharding(mesh, PartitionSpec("core"))
    donate = tuple(range(n_params, n_params + n_outs))
    sharded = jax.jit(
        shard_map(
            _body,
            mesh=mesh,
            in_specs=(PartitionSpec("core"),) * (n_params + n_outs),
            out_specs=(PartitionSpec("core"),) * n_outs,
            check_rep=False,
        ),
        donate_argnums=donate,
        keep_unused=True,
    )
    concat_in = [
        np.concatenate([np.asarray(m[name]) for m in in_maps], axis=0)
        for name in in_names
    ]
    dev_in = [jax.device_put(a, sharding) for a in concat_in]
    jax.block_until_ready(dev_in)

    times = []
    out_arrs = None
    for _ in range(max(1, reps)):
        dev_zero = [
            jax.device_put(
                np.zeros((_NC * av.shape[0], *av.shape[1:]), av.dtype), sharding
            )
            for av in out_avals
        ]
        jax.block_until_ready(dev_zero)
        t0 = time.perf_counter()
        out_arrs = sharded(*dev_in, *dev_zero)
        jax.block_until_ready(out_arrs)
        times.append(time.perf_counter() - t0)
    last_exec_times = times

    return [
        {
            name: np.asarray(out_arrs[i]).reshape(_NC, *out_avals[i].shape)[c]
            for i, name in enumerate(out_names)
        }
        for c in range(_NC)
    ]


def _gumbel_sampled(logits):
    """step < total_steps // 2 branch: reproduce the reference's Gumbel-max
    sampling exactly (needs jax's threefry on CPU, so run in a subprocess
    with JAX_PLATFORMS=cpu)."""
    import pickle
    import subprocess
    import sys
    import tempfile

    with tempfile.TemporaryDirectory() as td:
        lp = os.path.join(td, "l.npy")
        op = os.path.join(td, "o.npy")
        np.save(lp, logits)
        code = (
            "import numpy as np, jax, jax.numpy as jnp\n"
            f"l = jnp.asarray(np.load({lp!r}))\n"
            "g = -jnp.log(-jnp.log(jax.random.uniform(jax.random.key(1), l.shape) + 1e-20) + 1e-20)\n"
            f"np.save({op!r}, np.asarray(jnp.argmax(l + g, axis=-1)))\n"
        )
        env = dict(os.environ, JAX_PLATFORMS="cpu")
        subprocess.run([sys.executable, "-c", code], check=True, env=env)
        return np.load(op)


def kernel(logits, hidden_states, current_mask, W1, b1, W2, b2, step, total_steps):
    logits = np.asarray(logits, dtype=np.float32)
    hidden = np.asarray(hidden_states, dtype=np.float32)
    mask = np.asarray(current_mask).astype(bool)
    W1 = np.asarray(W1, dtype=np.float32)
    b1 = np.asarray(b1, dtype=np.float32)
    W2 = np.asarray(W2, dtype=np.float32)
    b2 = np.asarray(b2, dtype=np.float32)
    step_i = int(step)
    total_i = int(total_steps)

    B, S, V = logits.shape
    E = hidden.shape[-1]
    assert (B, S, V, E) == (_B, _S, _V, _E), "kernel compiled for fixed shapes"

    lg_flat = np.ascontiguousarray(logits.reshape(B * S, V))
    hd_flat = hidden.reshape(B * S, E)
    w1t = np.ascontiguousarray(W1.T)  # [E, F]
    w2t = np.ascontiguousarray(W2.reshape(-1))  # [F]

    in_maps = []
    for i in range(_NC):
        rows = slice(i * _R, (i + 1) * _R)
        in_maps.append(
            {
                "lg": lg_flat[rows],
                "ht": np.ascontiguousarray(hd_flat[rows].T),
                "w1t": w1t,
                "b1v": b1,
                "w2t": w2t,
            }
        )

    reps = int(os.environ.get("KERNEL_TIME_REPS", "1"))
    outs = _run_device(in_maps, reps=reps)

    sumexp = np.concatenate([o["o_sum"].reshape(-1) for o in outs])
    maxexp = np.concatenate([o["o_max"].reshape(-1) for o in outs])
    argv = np.concatenate([o["o_arg"].reshape(-1) for o in outs])
    x2 = np.concatenate([o["o_x2"].reshape(-1) for o in outs])

    # ---- O(B*S) epilogue, mirroring the reference in float32 ----
    max_prob = (maxexp / sumexp).astype(np.float32)
    z = (x2 + b2.reshape(-1)[0]).astype(np.float32)
    learned = np.float32(1.0) / (np.float32(1.0) + np.exp(-z, dtype=np.float32))
    mask_flat = mask.reshape(-1)
    conf = (np.float32(0.8) * max_prob + np.float32(0.2) * learned) * mask_flat
    conf = conf.astype(np.float32).reshape(B, S)

    above = mask & (conf > _THRESHOLD)
    any_above = above.any(axis=-1, keepdims=True)
    has_masked = mask.any(axis=-1, keepdims=True)
    masked_conf = np.where(mask, conf, -np.inf)
    best = masked_conf.argmax(axis=-1)
    fallback = (np.arange(S)[None, :] == best[:, None]) & has_masked
    unmask = np.where(any_above, above, fallback)
    new_mask = mask & ~unmask

    if step_i < total_i // 2:
        sampled = _gumbel_sampled(logits)
    else:
        sampled = np.rint(argv).astype(np.int32).reshape(B, S)
    unmasked_tokens = np.where(unmask, sampled, 0).astype(np.int32)

    return conf, new_mask, unmasked_tokens


# revision 11
# speedup vs baseline: 151.7517x; 151.7517x over previous
"""Trainium2 Bass kernel for ConfidenceMaskedDecoder.

Strategy (8 NeuronCores, data-parallel over the B*S=8192 rows, 1024 rows/core):
  Device, per core (rows r = token positions, V=32000 vocab, E=2048 hidden):
    * Stream logits [1024, 32000] f32 through SBUF in [128, 2000] chunks:
        - ACT: exp(chunk) -> bf16 tile, fused accumulate-sum -> per-row sumexp
        - DVE: per-chunk max of the bf16 exps (2x mode), then find-index of
          that max within the chunk (InstMaxIndex)
        - tiny per-group combines give per-row max(exp) and global argmax
      max softmax prob = max(exp(l)) / sum(exp(l)); no max-subtraction is
      needed (|logits| <= ~6 here so exp cannot overflow in fp32).
    * Confidence head on PE: out1^T[f, r] = W1^T.T @ hidden^T (accumulate over
      E in 16 K-chunks of 128), ACT Gelu(+b1) -> h^T, then
      x2[1, r] = W2^T.T @ h^T accumulated over the 8 f-chunks.
  Host: only O(B*S) epilogue — sigmoid, confidence mix, threshold/fallback
  mask update, token scatter — mirroring the reference in float32 numpy.
"""

import os
import time

import numpy as np

_P = 128
_B, _S, _V, _E = 4, 2048, 32000, 2048
_F = _E // 2  # 1024
_NC = 8  # cores
_RT = _B * _S  # 8192 rows total
_R = _RT // _NC  # 1024 rows per core
_G = _R // _P  # 8 row groups per core
_CV = 2000  # vocab chunk
_NCH = _V // _CV  # 16 chunks
_NR = 512  # rows per matmul tile (PSUM free dim)
_NN = _R // _NR  # 2
_KE = _E // _P  # 16 contraction chunks
_FC = _F // _P  # 8 feature chunks

_THRESHOLD = np.float32(0.8)

# fp32 matmuls: 4 cycles/row on PE but bit-accurate enough that the
# argmax-over-S of conf matches the fp32 reference.  "float32r" runs 4x
# faster; flipped on after HW validation (see MM_DTYPE_ENV).
_MM_DTYPE = os.environ.get("KERNEL_MM_DTYPE", "float32r")

_nc_cache = {}
last_exec_times = None  # list of per-rep seconds for the last device run


def _build_nc():
    import concourse.bacc as bacc
    import concourse.mybir as mybir
    import concourse.tile as tile

    f32 = mybir.dt.float32
    bf16 = mybir.dt.bfloat16
    u32 = mybir.dt.uint32
    mmdt = getattr(mybir.dt, _MM_DTYPE)
    AF = mybir.ActivationFunctionType
    ALU = mybir.AluOpType
    AX = mybir.AxisListType

    nc = bacc.Bacc("TRN2", target_bir_lowering=False, debug=False, num_devices=_NC)
    lg = nc.dram_tensor("lg", [_R, _V], f32, kind="ExternalInput").ap()
    ht = nc.dram_tensor("ht", [_E, _R], mmdt, kind="ExternalInput").ap()
    w1t = nc.dram_tensor("w1t", [_E, _F], mmdt, kind="ExternalInput").ap()
    b1v = nc.dram_tensor("b1v", [_F], f32, kind="ExternalInput").ap()
    w2t = nc.dram_tensor("w2t", [_F], mmdt, kind="ExternalInput").ap()
    o_sum = nc.dram_tensor("o_sum", [_G, _P], f32, kind="ExternalOutput").ap()
    o_max = nc.dram_tensor("o_max", [_G, _P], f32, kind="ExternalOutput").ap()
    o_arg = nc.dram_tensor("o_arg", [_G, _P], f32, kind="ExternalOutput").ap()
    o_x2 = nc.dram_tensor("o_x2", [1, _R], f32, kind="ExternalOutput").ap()

    with tile.TileContext(nc) as tc:
        with (
            tc.tile_pool(name="consts", bufs=1) as consts,
            tc.tile_pool(name="outacc", bufs=1) as outacc,
            tc.tile_pool(name="htp", bufs=2) as htp,
            tc.tile_pool(name="hgp", bufs=1) as hgp,
            tc.tile_pool(name="lgp", bufs=4) as lgp,
            tc.tile_pool(name="exq", bufs=4) as exq,
            tc.tile_pool(name="stats", bufs=2) as stats,
            tc.tile_pool(name="small", bufs=4) as small,
            tc.tile_pool(name="ps1", bufs=6, space="PSUM") as ps1p,
            tc.tile_pool(name="ps2", bufs=2, space="PSUM") as ps2p,
        ):
            # ---- replicated constants ----
            w1t_sb = consts.tile([_P, _KE, _F], mmdt)
            nc.sync.dma_start(out=w1t_sb[:], in_=w1t.rearrange("(k p) f -> p k f", p=_P))
            b1_sb = consts.tile([_P, _FC], f32)
            nc.sync.dma_start(out=b1_sb[:], in_=b1v.rearrange("(c p) -> p c", p=_P))
            w2t_sb = consts.tile([_P, _FC], mmdt)
            nc.sync.dma_start(out=w2t_sb[:], in_=w2t.rearrange("(c p) -> p c", p=_P))
            offs = consts.tile([_P, _NCH], f32)
            for c in range(_NCH):
                nc.vector.memset(offs[:, c : c + 1], float(c * _CV))
            bigc = consts.tile([_P, _NCH], f32)
            nc.vector.memset(bigc[:], 1.0e9)

            osum_sb = outacc.tile([_P, _G], f32)
            omax_sb = outacc.tile([_P, _G], f32)
            oarg_sb = outacc.tile([_P, _G], f32)
            x2_sb = outacc.tile([1, _R], f32)

            # ---- logits streaming: sumexp, max(exp), argmax ----
            for g in range(_G):
                sech = stats.tile([_P, _NCH], f32, tag="sech")
                cmax = stats.tile([_P, _NCH], bf16, tag="cmax")
                fsc = stats.tile([_P, _NCH, 8], u32, tag="fsc")
                for c in range(_NCH):
                    lt = lgp.tile([_P, _CV], f32, tag="lt")
                    nc.sync.dma_start(
                        out=lt[:], in_=lg[g * _P : (g + 1) * _P, c * _CV : (c + 1) * _CV]
                    )
                    et = exq.tile([_P, _CV], bf16, tag="et")
                    nc.scalar.activation(
                        out=et[:], in_=lt[:], func=AF.Exp, accum_out=sech[:, c : c + 1]
                    )
                    nc.vector.tensor_reduce(
                        out=cmax[:, c : c + 1], in_=et[:], axis=AX.X, op=ALU.max
                    )
                    nc.vector.max_index(
                        out=fsc[:, c, :],
                        in_max=cmax[:, c : c + 1].to_broadcast([_P, 8]),
                        in_values=et[:],
                    )
                gmaxb = small.tile([_P, 1], bf16, tag="gmaxb")
                nc.vector.tensor_reduce(out=gmaxb[:], in_=cmax[:], axis=AX.X, op=ALU.max)
                nc.gpsimd.tensor_copy(out=omax_sb[:, g : g + 1], in_=gmaxb[:])
                nc.vector.tensor_reduce(
                    out=osum_sb[:, g : g + 1], in_=sech[:], axis=AX.X, op=ALU.add
                )
                cidx = small.tile([_P, _NCH], f32, tag="cidx")
                nc.gpsimd.tensor_copy(out=cidx[:], in_=fsc[:, :, 0])
                nc.gpsimd.tensor_tensor(out=cidx[:], in0=cidx[:], in1=offs[:], op=ALU.add)
                eq = small.tile([_P, _NCH], u32, tag="eq")
                nc.vector.tensor_scalar(
                    out=eq[:],
                    in0=cmax[:],
                    scalar1=omax_sb[:, g : g + 1],
                    scalar2=None,
                    op0=ALU.is_equal,
                )
                cand = small.tile([_P, _NCH], f32, tag="cand")
                nc.vector.select(out=cand[:], mask=eq[:], on_true=cidx[:], on_false=bigc[:])
                nc.vector.tensor_reduce(
                    out=oarg_sb[:, g : g + 1], in_=cand[:], axis=AX.X, op=ALU.min
                )

            # ---- confidence-head MLP ----
            ht_r = ht.rearrange("(k p) r -> p k r", p=_P)
            for n in range(_NN):
                ht_t = htp.tile([_P, _KE, _NR], mmdt, tag="ht")
                nc.sync.dma_start(out=ht_t[:], in_=ht_r[:, :, n * _NR : (n + 1) * _NR])
                hg = hgp.tile([_P, _FC, _NR], mmdt, tag="hg")
                for fb in range(2):
                    pstiles = [
                        ps1p.tile([_P, _NR], f32, tag="ps1", name=f"ps1_{n}_{fb}_{i}")
                        for i in range(4)
                    ]
                    for ff in range(4):
                        fc = fb * 4 + ff
                        for k in range(_KE):
                            nc.tensor.matmul(
                                pstiles[ff][:],
                                lhsT=w1t_sb[:, k, fc * _P : (fc + 1) * _P],
                                rhs=ht_t[:, k, :],
                                start=(k == 0),
                                stop=(k == _KE - 1),
                            )
                    for ff in range(4):
                        fc = fb * 4 + ff
                        nc.scalar.activation(
                            out=hg[:, fc, :],
                            in_=pstiles[ff][:],
                            func=AF.Gelu,
                            bias=b1_sb[:, fc : fc + 1],
                            scale=1.0,
                        )
                ps2 = ps2p.tile([1, _NR], f32, tag="ps2")
                for fc in range(_FC):
                    nc.tensor.matmul(
                        ps2[:],
                        lhsT=w2t_sb[:, fc : fc + 1],
                        rhs=hg[:, fc, :],
                        start=(fc == 0),
                        stop=(fc == _FC - 1),
                    )
                nc.scalar.copy(out=x2_sb[0:1, n * _NR : (n + 1) * _NR], in_=ps2[:])

            nc.sync.dma_start(out=o_sum.rearrange("g p -> p g"), in_=osum_sb[:])
            nc.sync.dma_start(out=o_max.rearrange("g p -> p g"), in_=omax_sb[:])
            nc.sync.dma_start(out=o_arg.rearrange("g p -> p g"), in_=oarg_sb[:])
            nc.sync.dma_start(out=o_x2[:], in_=x2_sb[:])

    nc.compile()
    return nc


def _get_nc():
    if "nc" not in _nc_cache:
        _nc_cache["nc"] = _build_nc()
    return _nc_cache["nc"]


def _run_device(in_maps, reps=1):
    """Run the per-core kernel on the 8 NeuronCores.  Modeled on
    concourse.bass2jax.run_bass_via_pjrt, with input pre-staging so repeated
    executions time the NEFF itself rather than host->device transfer."""
    global last_exec_times
    import jax
    import concourse.mybir as mybir
    from jax.experimental.shard_map import shard_map
    from jax.sharding import Mesh, NamedSharding, PartitionSpec
    from concourse import bass2jax

    nc = _get_nc()
    bass2jax.install_neuronx_cc_hook()

    partition_name = nc.partition_id_tensor.name if nc.partition_id_tensor else None
    in_names, out_names, out_avals = [], [], []
    for alloc in nc.m.functions[0].allocations:
        if not isinstance(alloc, mybir.MemoryLocationSet):
            continue
        name = alloc.memorylocations[0].name
        if alloc.kind == "ExternalInput":
            if name != partition_name:
                in_names.append(name)
        elif alloc.kind == "ExternalOutput":
            out_names.append(name)
            out_avals.append(
                jax.core.ShapedArray(tuple(alloc.tensor_shape), mybir.dt.np(alloc.dtype))
            )
    n_params = len(in_names)
    n_outs = len(out_names)
    all_names = in_names + out_names
    if partition_name is not None:
        all_names = all_names + [partition_name]

    def _body(*args):
        operands = list(args)
        if partition_name is not None:
            operands.append(bass2jax.partition_id_tensor())
        outs = bass2jax._bass_exec_p.bind(
            *operands,
            out_avals=tuple(out_avals),
            in_names=tuple(all_names),
            out_names=tuple(out_names),
            lowering_input_output_aliases=(),
            sim_require_finite=True,
            sim_require_nnan=True,
            nc=nc,
        )
        return tuple(outs)

    devices = jax.devices()[:_NC]
    mesh = Mesh(np.asarray(devices), ("core",))
    sharding = NamedSharding(mesh, PartitionSpec("core"))
    donate = tuple(range(n_params, n_params + n_outs))
    sharded = jax.jit(
        shard_map(
            _body,
            mesh=mesh,
            in_specs=(PartitionSpec("core"),) * (n_params + n_outs),
            out_specs=(PartitionSpec("core"),) * n_outs,
            check_rep=False,
        ),
        donate_argnums=donate,
        keep_unused=True,
    )
    concat_in = [
        np.concatenate([np.asarray(m[name]) for m in in_maps], axis=0)
        for name in in_names
    ]
    dev_in = [jax.device_put(a, sharding) for a in concat_in]
    jax.block_until_ready(dev_in)

    times = []
    out_arrs = None
    for _ in range(max(1, reps)):
        dev_zero = [
            jax.device_put(
                np.zeros((_NC * av.shape[0], *av.shape[1:]), av.dtype), sharding
            )
            for av in out_avals
        ]
        jax.block_until_ready(dev_zero)
        t0 = time.perf_counter()
        out_arrs = sharded(*dev_in, *dev_zero)
        jax.block_until_ready(out_arrs)
        times.append(time.perf_counter() - t0)
    last_exec_times = times

    return [
        {
            name: np.asarray(out_arrs[i]).reshape(_NC, *out_avals[i].shape)[c]
            for i, name in enumerate(out_names)
        }
        for c in range(_NC)
    ]


def _gumbel_sampled(logits):
    """step < total_steps // 2 branch: reproduce the reference's Gumbel-max
    sampling exactly (needs jax's threefry on CPU, so run in a subprocess
    with JAX_PLATFORMS=cpu)."""
    import pickle
    import subprocess
    import sys
    import tempfile

    with tempfile.TemporaryDirectory() as td:
        lp = os.path.join(td, "l.npy")
        op = os.path.join(td, "o.npy")
        np.save(lp, logits)
        code = (
            "import numpy as np, jax, jax.numpy as jnp\n"
            f"l = jnp.asarray(np.load({lp!r}))\n"
            "g = -jnp.log(-jnp.log(jax.random.uniform(jax.random.key(1), l.shape) + 1e-20) + 1e-20)\n"
            f"np.save({op!r}, np.asarray(jnp.argmax(l + g, axis=-1)))\n"
        )
        env = dict(os.environ, JAX_PLATFORMS="cpu")
        subprocess.run([sys.executable, "-c", code], check=True, env=env)
        return np.load(op)


def kernel(logits, hidden_states, current_mask, W1, b1, W2, b2, step, total_steps):
    logits = np.asarray(logits, dtype=np.float32)
    hidden = np.asarray(hidden_states, dtype=np.float32)
    mask = np.asarray(current_mask).astype(bool)
    W1 = np.asarray(W1, dtype=np.float32)
    b1 = np.asarray(b1, dtype=np.float32)
    W2 = np.asarray(W2, dtype=np.float32)
    b2 = np.asarray(b2, dtype=np.float32)
    step_i = int(step)
    total_i = int(total_steps)

    B, S, V = logits.shape
    E = hidden.shape[-1]
    assert (B, S, V, E) == (_B, _S, _V, _E), "kernel compiled for fixed shapes"

    lg_flat = np.ascontiguousarray(logits.reshape(B * S, V))
    hd_flat = hidden.reshape(B * S, E)
    w1t = np.ascontiguousarray(W1.T)  # [E, F]
    w2t = np.ascontiguousarray(W2.reshape(-1))  # [F]

    in_maps = []
    for i in range(_NC):
        rows = slice(i * _R, (i + 1) * _R)
        in_maps.append(
            {
                "lg": lg_flat[rows],
                "ht": np.ascontiguousarray(hd_flat[rows].T),
                "w1t": w1t,
                "b1v": b1,
                "w2t": w2t,
            }
        )

    reps = int(os.environ.get("KERNEL_TIME_REPS", "1"))
    outs = _run_device(in_maps, reps=reps)

    sumexp = np.concatenate([o["o_sum"].reshape(-1) for o in outs])
    maxexp = np.concatenate([o["o_max"].reshape(-1) for o in outs])
    argv = np.concatenate([o["o_arg"].reshape(-1) for o in outs])
    x2 = np.concatenate([o["o_x2"].reshape(-1) for o in outs])

    # ---- O(B*S) epilogue, mirroring the reference in float32 ----
    max_prob = (maxexp / sumexp).astype(np.float32)
    z = (x2 + b2.reshape(-1)[0]).astype(np.float32)
    learned = np.float32(1.0) / (np.float32(1.0) + np.exp(-z, dtype=np.float32))
    mask_flat = mask.reshape(-1)
    conf = (np.float32(0.8) * max_prob + np.float32(0.2) * learned) * mask_flat
    conf = conf.astype(np.float32).reshape(B, S)

    above = mask & (conf > _THRESHOLD)
    any_above = above.any(axis=-1, keepdims=True)
    has_masked = mask.any(axis=-1, keepdims=True)
    masked_conf = np.where(mask, conf, -np.inf)
    best = masked_conf.argmax(axis=-1)
    fallback = (np.arange(S)[None, :] == best[:, None]) & has_masked
    unmask = np.where(any_above, above, fallback)
    new_mask = mask & ~unmask

    if step_i < total_i // 2:
        sampled = _gumbel_sampled(logits)
    else:
        sampled = np.rint(argv).astype(np.int32).reshape(B, S)
    unmasked_tokens = np.where(unmask, sampled, 0).astype(np.int32)

    return conf, new_mask, unmasked_tokens
